# revision 4
# baseline (speedup 1.0000x reference)
"""CLOULoss Trainium2 kernel, v3 (latency-optimized).

loss = (term1 - term2) / (B*(C-1)^2), term1 via a degree-(N-1) Newton
interpolant of F(p) = sum_{k!=l} softplus(p - t_kl).

v3 structure (per core):
- two input DMAs: a small one with the y data + matmul helper rows
  (lands first), a second with scan/mask/grid constants (needed later).
- distances: d2 = -2*y^T y + n_k + n_l with the eps terms dropped
  (|delta d2| ~ 4e-5 -> ~1e-6 in the loss); the k==l / i==j diagonals
  are exactly recovered by the CEPS2 clamp + compile-time constants.
  d2t is produced directly in [128, 32] (two tile_position halves),
  the norm row n comes from one ones^T @ sq matmul (partition 0 for
  the row form, an M=2 [0|1] weight column writes partition 1), and
  each half adds both rank-1 terms with a single K=2 matmul against
  [n; ones] / [ones; n] zones assembled in the input tile.
- node sums: W[p, 32r+j] = e^{nu_r} * E[p, j] via one stride-0 DVE
  mult, one fat Ln(W+1) ACT over [128, 256], one grouped DVE reduce
  to [128, 8] (block r=7 is nu=0 for the i==j diagonal F(0) term).
- Newton coeffs via two tiny matmuls, broadcast as two M=64 halves,
  the i==j mask folded into the data1 build, one tensor_tensor_scan,
  and three accumulating matmuls for the final scalar.  The
  compile-time cc constant is added on the host after the 8-core sum.
"""

import numpy as np

B = 128
C = 64
EPS = 1e-6
N_CORES = 8
RPC = B // N_CORES          # 16
N_NODES = 7
N_BLK = N_NODES
SCAN_W = RPC * N_BLK        # 112
P_LO, P_HI = 7.6, 15.2
DENOM = float(B * (C - 1) ** 2)
T_DIAG = 8e-6
CEPS2 = float(C) * EPS * EPS
NG = N_NODES + 1            # grid blocks (incl nu=0)

# d1 layout ([64, 512] f32): y data + helper zones
D1_YT = 0       # [0:64)    ytt   (rows 0:64)
D1_YR = 64      # [64:80)   yrt
D1_YP = 80      # [80:208)  ypt
D1_OC = 208     # [208:209) ones column (rows 0:66)
D1_ZO = 209     # [209:211) M=2 weight cols [0 | 1] (rows 0:64)
D1_NR = 224     # [224:432) row0 = n row (device), row1 = ones (host)
D1_NR2 = 432    # [432:512) row0 = ones (host), row1 = [n_t|n_r] (device)
D1_W = 512

# d2 layout ([128, 704] f32): constants needed later
D2_EX = 0       # [0:256)   expnu_ext: [p, 32r+j] = e^{nu_r}
D2_NU = 256     # [256:368) nu_ext
D2_SEL = 368    # [368:480) sel_ext
D2_M01 = 480    # [480:512) mask01 in [128,32] layout
D2_M16 = 512    # [512:528) per-core diag mask
D2_OC = 528     # [528:529) ones column (rows 0:128)
D2_W1 = 529
D2_W2 = 530
D2_W4 = 531
D2_M2 = 532     # [532:539) Marev^T (rows 0:7)
D2_MC = 539     # [539:546) -(Marev@corr) row (partition 0)
D2_OR = 546     # [546:674) ones row (partition 0)
D2_W = 704

_CONSTS = None
_PROGS = {}


def _softplus64(x):
    return np.logaddexp(0.0, np.asarray(x, dtype=np.float64))


def _host_consts():
    global _CONSTS
    if _CONSTS is not None:
        return _CONSTS
    n = N_NODES
    kk = np.arange(n)
    cheb = (P_LO + P_HI) / 2 + (P_HI - P_LO) / 2 * np.cos(
        np.pi * (2 * kk + 1) / (2 * n))
    pts = list(cheb)
    i0 = max(range(len(pts)), key=lambda i: abs(pts[i] - (P_LO + P_HI) / 2))
    order = [pts[i0]]
    del pts[i0]
    while pts:
        prods = [np.prod([abs(q - o) for o in order]) for q in pts]
        i = int(np.argmax(prods))
        order.append(pts[i])
        del pts[i]
    nodes = np.array(order)

    M0 = np.zeros((n, n))
    for e in range(n):
        a = np.zeros(n)
        a[e] = 1.0
        for j in range(1, n):
            a[j:] = (a[j:] - a[j - 1:-1]) / (nodes[j:] - nodes[:n - j])
        M0[:, e] = a
    S = np.diag((-1.0) ** np.arange(n))
    Marev = (S @ M0)[::-1]

    blk_nu = np.zeros(N_BLK)
    blk_sel = np.zeros(N_BLK)
    blk_nu[1:] = nodes[n - 2::-1]
    blk_sel[1:] = 1.0
    nu_ext = np.tile(np.tile(blk_nu, RPC)[None, :], (128, 1))
    sel_ext = np.tile(np.tile(blk_sel, RPC)[None, :], (128, 1))

    corr = float(C) * _softplus64(nodes - T_DIAG)
    neg_mcorr = -(Marev @ corr)
    cc_total = -float(C) * float(_softplus64(0.0 - T_DIAG)) * B / DENOM

    m01 = np.ones((128, 32), dtype=np.float32)
    for q in range(32):
        m01[q, q] = 0.0
    for q in range(96, 128):
        m01[q, q - 96] = 0.0

    expnu = np.exp(nodes)
    expnu_ext = np.zeros((128, 32 * NG), dtype=np.float32)
    for r in range(N_NODES):
        expnu_ext[:, 32 * r:32 * r + 32] = expnu[r]
    expnu_ext[:, 32 * N_NODES:32 * NG] = 1.0

    d1c = np.zeros((64, D1_W), dtype=np.float32)
    d1c[0:64, D1_OC] = 1.0
    d1c[0, D1_NR2:D1_NR2 + 80] = 1.0

    d2c = np.zeros((128, D2_W), dtype=np.float32)
    d2c[:, D2_EX:D2_EX + 32 * NG] = expnu_ext
    d2c[:, D2_NU:D2_NU + SCAN_W] = nu_ext
    d2c[:, D2_SEL:D2_SEL + SCAN_W] = sel_ext
    d2c[:, D2_M01:D2_M01 + 32] = m01
    d2c[:, D2_OC] = 1.0
    d2c[:, D2_W1] = 1.0 / DENOM
    d2c[:, D2_W2] = -(B * B / float(N_CORES)) / DENOM
    d2c[:, D2_W4] = RPC / DENOM
    d2c[0:n, D2_M2:D2_M2 + n] = Marev.T
    d2c[0, D2_MC:D2_MC + n] = neg_mcorr
    d2c[0, D2_OR:D2_OR + 128] = 1.0

    masks16 = []
    for c in range(N_CORES):
        m = np.ones((128, RPC), dtype=np.float32)
        for i in range(RPC):
            m[RPC * c + i, i] = 0.0
        masks16.append(m)

    _CONSTS = dict(nodes=nodes, d1c=d1c, d2c=d2c, masks16=masks16,
                   cc_total=cc_total)
    return _CONSTS


def _fix_act_table_loads(nc, mybir):
    from concourse.hw_specs import get_activation_tables
    names = list(get_activation_tables(nc.m.arch).keys())
    both_id = names.index("natural_log_exp_and_others")
    first = True
    for b in nc.main_func.blocks:
        keep = []
        for i in b.instructions:
            if isinstance(i, mybir.InstLoadActFuncSet):
                si = i.sync_info
                assert si is None or (not si.on_wait and not si.on_update)
                if first:
                    i.act_func_set_id = both_id
                    first = False
                    keep.append(i)
            else:
                keep.append(i)
        b.instructions[:] = keep


def _build_program():
    if None in _PROGS:
        return _PROGS[None]
    import concourse.bass as bass
    import concourse.bacc as bacc
    import concourse.mybir as mybir
    from concourse import tile

    AF = mybir.ActivationFunctionType
    OP = mybir.AluOpType
    f32 = mybir.dt.float32
    n = N_NODES

    nc = bacc.Bacc("TRN2", target_bir_lowering=False, debug=False,
                   num_devices=N_CORES)

    d1_d = nc.dram_tensor("d1", [64, D1_W], f32, kind="ExternalInput").ap()
    d2_d = nc.dram_tensor("d2", [128, D2_W], f32, kind="ExternalInput").ap()
    o_d = nc.dram_tensor("o", [1, 1], f32, kind="ExternalOutput").ap()

    with tile.TileContext(nc) as tc:
        with tc.tile_pool(name="sb", bufs=1) as sb:
            d1 = sb.tile([64, D1_W], f32)
            nc.sync.dma_start(d1[:], d1_d[:])
            d2 = sb.tile([128, D2_W], f32)
            # separate queue: keeps d1's completion ahead of this transfer
            nc.gpsimd.dma_start(d2[:], d2_d[:])

            Y = d1[0:64, D1_YT:D1_YT + 208]
            ytt = d1[0:64, D1_YT:D1_YT + 64]
            yrt = d1[0:64, D1_YR:D1_YR + 16]
            onec64 = d1[0:64, D1_OC:D1_OC + 1]

            ex_ext = d2[:, D2_EX:D2_EX + 32 * NG]
            nu_ext = d2[:, D2_NU:D2_NU + SCAN_W]
            sel_ext = d2[:, D2_SEL:D2_SEL + SCAN_W]
            m01 = d2[:, D2_M01:D2_M01 + 32]
            m16 = d2[:, D2_M16:D2_M16 + RPC]
            onec = d2[:, D2_OC:D2_OC + 1]
            w1 = d2[:, D2_W1:D2_W1 + 1]
            w2 = d2[:, D2_W2:D2_W2 + 1]
            w4 = d2[:, D2_W4:D2_W4 + 1]
            m2 = d2[0:n, D2_M2:D2_M2 + n]
            mcorr = d2[0:1, D2_MC:D2_MC + n]
            oner = d2[0:1, D2_OR:D2_OR + 128]
            one11 = d2[0:1, D2_OC:D2_OC + 1]

            # prep
            n2 = sb.tile([64, 208], f32)
            nc.vector.tensor_scalar(n2[:], Y, -2.0, None, OP.mult)
            sq = sb.tile([64, 208], f32)
            nc.vector.tensor_tensor(sq[:], Y, Y, OP.mult)

            with tc.tile_pool(name="ps", bufs=1, space="PSUM") as ps:
                # norm row n = ones^T @ sq -> partition 0, copied into d1.
                # Split so the d2t rank-1s only wait on the [n_t|n_r] half.
                rows_ps = ps.tile([1, 208], f32)
                nc.tensor.matmul(rows_ps[:, 0:80], onec64, sq[:, 0:80],
                                 start=True, stop=True)
                nc.vector.tensor_copy(d1[0:1, D1_NR:D1_NR + 80],
                                      rows_ps[:, 0:80])
                nc.tensor.matmul(rows_ps[:, 80:208], onec64, sq[:, 80:208],
                                 start=True, stop=True)
                nc.vector.tensor_copy(d1[0:1, D1_NR + 80:D1_NR + 208],
                                      rows_ps[:, 80:208])
                nrow = d1[0:1, D1_NR:D1_NR + 208]
                rz0 = d1[0:1, D1_NR2:D1_NR2 + 80]   # host ones row

                # d2t in [128, 32] (two halves), d2p in [128, 16]
                d2t_ps = ps.tile([128, 32], f32)
                nc.tensor.matmul(d2t_ps[0:64, :], n2[:, 0:64], ytt[:, 0:32],
                                 start=True, stop=False)
                nc.tensor.matmul(d2t_ps[0:64, :], nrow[:, 0:64],
                                 rz0[:, 0:32], start=False, stop=False)
                nc.tensor.matmul(d2t_ps[0:64, :], rz0[:, 0:64],
                                 nrow[:, 0:32], start=False, stop=True)
                nc.tensor.matmul(d2t_ps[64:128, :], n2[:, 0:64],
                                 ytt[:, 32:64], start=True, stop=False,
                                 tile_position=(0, 64))
                nc.tensor.matmul(d2t_ps[64:128, :], nrow[:, 0:64],
                                 rz0[:, 0:32], start=False, stop=False,
                                 tile_position=(0, 64))
                nc.tensor.matmul(d2t_ps[64:128, :], rz0[:, 0:64],
                                 nrow[:, 32:64], start=False, stop=True,
                                 tile_position=(0, 64))
                d2p_ps = ps.tile([128, 16], f32)
                nc.tensor.matmul(d2p_ps[0:64, :], n2[:, 80:144], yrt,
                                 start=True, stop=False)
                nc.tensor.matmul(d2p_ps[0:64, :], nrow[:, 80:144],
                                 rz0[:, 0:16], start=False, stop=False)
                nc.tensor.matmul(d2p_ps[0:64, :], rz0[:, 0:64],
                                 nrow[:, 64:80], start=False, stop=True)
                nc.tensor.matmul(d2p_ps[64:128, :], n2[:, 144:208], yrt,
                                 start=True, stop=False,
                                 tile_position=(0, 64))
                nc.tensor.matmul(d2p_ps[64:128, :], nrow[:, 144:208],
                                 rz0[:, 0:16], start=False, stop=False,
                                 tile_position=(0, 64))
                nc.tensor.matmul(d2p_ps[64:128, :], rz0[:, 0:64],
                                 nrow[:, 64:80], start=False, stop=True,
                                 tile_position=(0, 64))

                d2t = sb.tile([128, 32], f32)
                nc.vector.tensor_scalar(d2t[:], d2t_ps[:], CEPS2, None,
                                        OP.max)
                d2p = sb.tile([128, 16], f32)
                nc.vector.tensor_scalar(d2p[:], d2p_ps[:], CEPS2, None,
                                        OP.max)

                # ACT chain
                lnt = sb.tile([128, 32], f32)
                nc.scalar.activation(lnt[:], d2t[:], AF.Ln)
                t_sb = sb.tile([128, 32], f32)
                nc.scalar.activation(t_sb[:], lnt[:], AF.Exp, scale=0.5)
                e_sb = sb.tile([128, 32], f32)
                nc.scalar.activation(e_sb[:], t_sb[:], AF.Exp, scale=-1.0)
                lnp = sb.tile([128, 16], f32)
                nc.scalar.activation(lnp[:], d2p[:], AF.Ln)
                p_sb = sb.tile([128, 16], f32)
                nc.scalar.activation(p_sb[:], lnp[:], AF.Exp, scale=0.5)

                # node grid W = e (x) expnu, one fat Ln, grouped reduce
                W = sb.tile([128, 32 * NG], f32)
                esl = e_sb[:]
                e_rep = bass.AP(esl.tensor, esl.offset,
                                [[esl.ap[0][0], 128], [0, NG], [1, 32]])
                nc.vector.tensor_tensor(W[:], e_rep, ex_ext, OP.mult)
                spn = sb.tile([128, 32 * NG], f32)
                nc.scalar.activation(spn[:], W[:], AF.Ln, bias=1.0)
                acc = sb.tile([128, NG], f32)
                spn_v = spn[:].rearrange("p (r j) -> p r j", j=32)
                nc.vector.tensor_reduce(acc[:], spn_v,
                                        mybir.AxisListType.X, OP.add)

                # term2 (off critical path)
                tmask = sb.tile([128, 32], f32)
                nc.gpsimd.tensor_tensor(tmask[:], t_sb[:], m01, OP.mult)
                tsum = sb.tile([128, 1], f32)
                nc.vector.tensor_reduce(tsum[:], tmask[:],
                                        mybir.AxisListType.X, OP.add)

                # data0 = nu - sel*p (during the node phase)
                p_masked = sb.tile([128, SCAN_W], f32)
                pm_v = p_masked[:].rearrange("p (a b) -> p a b", b=N_BLK)
                sel_v = sel_ext.rearrange("p (a b) -> p a b", b=N_BLK)
                psl = p_sb[:]
                p_rep = bass.AP(psl.tensor, psl.offset,
                                [[psl.ap[0][0], 128], [1, RPC], [0, N_BLK]])
                nc.gpsimd.tensor_tensor(pm_v, sel_v, p_rep, OP.mult)
                data0 = sb.tile([128, SCAN_W], f32)
                nc.gpsimd.tensor_tensor(data0[:], nu_ext, p_masked[:],
                                        OP.subtract)

                # S -> Newton coeffs -> broadcast
                s_ps = ps.tile([n, 1], f32)
                nc.tensor.matmul(s_ps[:], acc[:, 0:n], onec, start=True,
                                 stop=True)
                s_sb = sb.tile([n, 1], f32)
                nc.vector.tensor_copy(s_sb[:], s_ps[:])
                arev_ps = ps.tile([1, n], f32)
                nc.tensor.matmul(arev_ps[:], s_sb[:], m2, start=True,
                                 stop=False)
                nc.tensor.matmul(arev_ps[:], one11, mcorr, start=False,
                                 stop=True)
                arev_sb = sb.tile([1, n], f32)
                nc.vector.tensor_copy(arev_sb[:], arev_ps[:])
                bc_ps = ps.tile([128, n], f32)
                nc.tensor.matmul(bc_ps[0:64, :], oner[:, 0:64], arev_sb[:],
                                 start=True, stop=True)
                nc.tensor.matmul(bc_ps[64:128, :], oner[:, 0:64],
                                 arev_sb[:], start=True, stop=True,
                                 tile_position=(0, 64))

                data1 = sb.tile([128, SCAN_W], f32)
                d1_v = data1[:].rearrange("p (a b) -> p a b", b=N_BLK)
                bc = bc_ps[:]
                bc_rep = bass.AP(bc.tensor, bc.offset,
                                 [[bc.ap[0][0], 128], [0, RPC], [1, N_BLK]])
                m16_rep = bass.AP(m16.tensor, m16.offset,
                                  [[m16.ap[0][0], 128], [1, RPC],
                                   [0, N_BLK]])
                nc.vector.tensor_tensor(d1_v, bc_rep, m16_rep, OP.mult)

                scan_out = sb.tile([128, SCAN_W], f32)
                nc.vector.tensor_tensor_scan(scan_out[:], data0[:],
                                             data1[:], 0.0, OP.mult, OP.add)
                fsum = sb.tile([128, 1], f32)
                nc.vector.tensor_reduce(fsum[:],
                                        scan_out[:, N_BLK - 1::N_BLK],
                                        mybir.AxisListType.X, OP.add)

                o_ps = ps.tile([1, 1], f32)
                nc.tensor.matmul(o_ps[:], fsum[:], w1, start=True,
                                 stop=False)
                nc.tensor.matmul(o_ps[:], tsum[:], w2, start=False,
                                 stop=False)
                nc.tensor.matmul(o_ps[:], acc[:, n:n + 1], w4, start=False,
                                 stop=True)
                o_sb = sb.tile([1, 1], f32)
                nc.vector.tensor_copy(o_sb[:], o_ps[:])
                nc.sync.dma_start(o_d[:], o_sb[:])

    nc.compile()
    _fix_act_table_loads(nc, mybir)
    _PROGS[None] = nc
    return nc


def _in_maps(y_pred, y_true):
    cst = _host_consts()
    y_pred = np.ascontiguousarray(y_pred, dtype=np.float32)
    y_true = np.ascontiguousarray(y_true, dtype=np.float32)
    d1 = cst["d1c"].copy()
    d1[0:64, D1_YT:D1_YT + 64] = y_true[:C].T
    d1[0:64, D1_YP:D1_YP + 128] = y_pred.T
    maps = []
    for c in range(N_CORES):
        d1c = d1.copy()
        d1c[0:64, D1_YR:D1_YR + RPC] = y_pred[RPC * c:RPC * (c + 1)].T
        d2c = cst["d2c"].copy()
        d2c[:, D2_M16:D2_M16 + RPC] = cst["masks16"][c]
        maps.append({"d1": d1c, "d2": d2c})
    return maps


def kernel(y_pred, y_true):
    from concourse import bass_utils
    cst = _host_consts()
    nc = _build_program()
    maps = _in_maps(y_pred, y_true)
    res = bass_utils.run_bass_kernel_spmd(nc, maps,
                                          core_ids=list(range(N_CORES)))
    total = cst["cc_total"]
    for r in res.results:
        total += float(r["o"][0, 0])
    return np.array([total], dtype=np.float32)
